# revision 23
# baseline (speedup 1.0000x reference)
"""Trainium2 Bass kernel for nn_MultiHeadAttention_85375359909998.

Causal MHA with (non-standard interleaved) RoPE, fp32 in/out.
  B=2, T=2048, D=1024, H=16, DH=64.

Sharding over 8 NeuronCores: data-parallel over batch (2) x tensor-parallel
over head groups (16 heads -> 4 groups of 4). Each core computes its batch's
QKV projection for its 4 heads, RoPE, causal attention, and a partial output
projection; the host sums the 4 partial projections per batch (the
"all-reduce") and concatenates batches.

Device-side layout notes (per core, heads grouped in pairs):
  - everything that feeds the PE is float32r (TF32-like, full-rate matmul)
  - q/k are produced *transposed* ([dh, t]) directly by the projection
    (host passes x^T and W^T); RoPE rotate-half is a 128x128 block-diagonal
    permutation matrix applied on the PE, combined with cos/sin on the DVE
  - scores are computed transposed (S^T[s, t]) so the A@V matmul can use
    P^T tiles as the moving operand with V ([s, dh]) stationary; V gets an
    appended ones-column so row-sums (softmax denominators) fall out of the
    same matmul; causal masking = skipping upper tiles + PE-adding a
    precomputed -inf pattern to the 4 diagonal blocks of each row strip
  - softmax normalization: reciprocal of the sums row, broadcast across
    partitions with a K=1 ones outer-product on the PE, multiplied on DVE.
"""

import sys
from contextlib import ExitStack

import numpy as np

try:
    import concourse.bass as bass  # noqa: F401
except ImportError:  # pragma: no cover
    sys.path.insert(0, "/opt/trn_rl_repo")
    import concourse.bass as bass  # noqa: F401

import concourse.tile as tile
from concourse import bacc, mybir
from concourse import bass_utils

B, T, D, H, DH = 2, 2048, 1024, 16, 64
NCORES = 8
GROUPS = 4          # head groups (tensor-parallel dimension)
HPC = H // GROUPS   # 4 heads per core
NPAIR = HPC // 2    # head pairs per core
TC512 = T // 512    # 4
SC128 = T // 128    # 16
KC = D // 128       # 8 contraction chunks for the projections
NEG8 = np.float32(-4.0e4)  # pre-scale additive mask: exp(NEG8/8) == 0, fits fp16

f32 = mybir.dt.float32
f32r = mybir.dt.float32r
f16 = mybir.dt.float16
EXP = mybir.ActivationFunctionType.Exp
COPY = mybir.ActivationFunctionType.Copy

PAT_OFF = [0, 128, 384, 768]
PAT_W = [128, 256, 384, 512]

_CACHE = {}


def _rope_tables():
    """cos/sin tables, transposed & stacked for the [2*64, t] chunk layout."""
    inv = 1.0 / (10000.0 ** (np.arange(0, DH, 2, dtype=np.float64) / DH))  # 32
    t = np.arange(T, dtype=np.float64)
    freqs = t[:, None] * inv[None, :]                 # [T, 32]
    emb = np.concatenate([freqs, freqs], axis=-1)     # [T, 64]
    cos = np.cos(emb).astype(np.float32).T            # [64, T]
    sin = np.sin(emb).astype(np.float32).T
    csc = np.concatenate([cos, cos], axis=0)          # [128, T]
    csn = np.concatenate([sin, sin], axis=0)
    return (np.ascontiguousarray(csc.astype(np.float16)),
            np.ascontiguousarray(csn.astype(np.float16)))


def _rot_matrix():
    """R.T for rotate_half: (R@v)[2i] = -v[2i+1], (R@v)[2i+1] = v[2i]."""
    R = np.zeros((DH, DH), dtype=np.float32)
    for i in range(DH // 2):
        R[2 * i, 2 * i + 1] = -1.0
        R[2 * i + 1, 2 * i] = 1.0
    R128 = np.zeros((128, 128), dtype=np.float32)
    R128[:DH, :DH] = R
    R128[DH:, DH:] = R
    return np.ascontiguousarray(R128.T)


def _diag_patterns():
    """Concatenated diagonal-block causal masks, widths 128/256/384/512."""
    blocks = []
    for r in range(4):
        w = 128 * (r + 1)
        sp = np.arange(128)[:, None]
        tp = np.arange(w)[None, :]
        blocks.append(np.where(tp < 128 * r + sp, NEG8, np.float32(0.0)))
    return np.ascontiguousarray(
        np.concatenate(blocks, axis=1).astype(np.float16))


def _emit(nc, tc, d, ctx):
    const = ctx.enter_context(tc.tile_pool(name="const", bufs=1))
    qkp = ctx.enter_context(tc.tile_pool(name="qk", bufs=1))
    vtp = ctx.enter_context(tc.tile_pool(name="vt", bufs=1))
    vp = ctx.enter_context(tc.tile_pool(name="v", bufs=1))
    att = ctx.enter_context(tc.tile_pool(name="att", bufs=1))
    ptp = ctx.enter_context(tc.tile_pool(name="pt", bufs=4))
    tmp = ctx.enter_context(tc.tile_pool(name="tmp", bufs=3))
    small = ctx.enter_context(tc.tile_pool(name="small", bufs=2))
    stage = ctx.enter_context(tc.tile_pool(name="stage", bufs=4))

    # ---- constants ----
    wqk_t, wv_t = [], []
    for kc in range(KC):
        w1 = const.tile([128, 512], f16, tag=f"wqk{kc}")
        nc.scalar.dma_start(w1[:], d["wqk"][128 * kc:128 * (kc + 1), :])
        wqk_t.append(w1)
        w2 = const.tile([128, 256], f16, tag=f"wv{kc}")
        nc.scalar.dma_start(w2[:], d["wv"][128 * kc:128 * (kc + 1), :])
        wv_t.append(w2)
    rT_t = const.tile([128, 128], f16, tag="rT")
    nc.scalar.dma_start(rT_t[:], d["rT"][:])
    csc_t = const.tile([128, T], f16, tag="csc")
    csn_t = const.tile([128, T], f16, tag="csn")
    id_t = const.tile([128, 128], f16, tag="ident")
    oc_t = const.tile([128, 1], f16, tag="onesc")
    pat_t = const.tile([128, 1280], f16, tag="pat")
    wp_t = [const.tile([128, D], f16, tag=f"wp{kc}", name=f"wp{kc}")
            for kc in range(2)]

    # ---- persistent activations ----
    xT_t = []
    for kc in range(KC):
        xt_ = const.tile([128, T], f16, tag=f"xt{kc}", name=f"xt{kc}")
        dma_eng = nc.sync if kc % 2 == 0 else nc.scalar
        dma_eng.dma_start(xt_[:], d["xT"][128 * kc:128 * (kc + 1), :])
        xT_t.append(xt_)

    qk = [qkp.tile([128, T], f16, tag=f"qk{j}", name=f"qk{j}")
          for j in range(4)]
    vt = [vtp.tile([128, T], f16, tag=f"vt{j}", name=f"vt{j}")
          for j in range(2)]
    v_sb = [vp.tile([128, 4 * 65], f16, tag=f"v{i}", name=f"v{i}")
            for i in range(SC128)]
    attn = [att.tile([128, T], f16, tag=f"at{p}", name=f"at{p}")
            for p in range(NPAIR)]

    # ===== phase 1: qkv projections (transposed) + RoPE + V transpose ==
    with tc.tile_pool(name="psA", bufs=4, space="PSUM") as psA, \
         tc.tile_pool(name="psR", bufs=2, space="PSUM") as psR, \
         tc.tile_pool(name="psT", bufs=2, space="PSUM") as psT:
        for tcc in range(TC512):
            tsl = slice(512 * tcc, 512 * (tcc + 1))
            xst = [xT_t[kc][:, tsl] for kc in range(KC)]
            if tcc == 0:
                # lower-priority constants: needed only from the RoPE /
                # transpose / attention / proj stages onward
                nc.gpsimd.dma_start(csc_t[:], d["csc"][:])
                nc.gpsimd.dma_start(csn_t[:], d["csn"][:])
                nc.gpsimd.dma_start(id_t[:], d["ident"][:])
                nc.gpsimd.dma_start(oc_t[:], d["onesc"][:])
            if tcc == 1:
                nc.gpsimd.dma_start(pat_t[:], d["pat"][:])
                for kc2 in range(2):
                    nc.gpsimd.dma_start(
                        wp_t[kc2][:], d["wp"][128 * kc2:128 * (kc2 + 1), :])
            for jp in range(3):
                pss2 = [psA.tile([128, 512], f32, tag="ps", name=f"ps{jc}")
                        for jc in (2 * jp, 2 * jp + 1)]
                for kc in range(KC):
                    for u in range(2):
                        jc = 2 * jp + u
                        lhsT = (wqk_t[kc][:, 128 * jc:128 * (jc + 1)]
                                if jc < 4 else
                                wv_t[kc][:, 128 * (jc - 4):128 * (jc - 3)])
                        nc.tensor.matmul(pss2[u][:], lhsT, xst[kc],
                                         start=(kc == 0), stop=(kc == KC - 1))
                for u in range(2):
                    jc = 2 * jp + u
                    ps = pss2[u]
                    if jc < 4:
                        dst = qk[jc][:, tsl]
                        nc.scalar.copy(dst, ps[:])
                        rps = psR.tile([128, 512], f32, tag="rps")
                        nc.tensor.matmul(rps[:], rT_t[:], dst,
                                         start=True, stop=True)
                        t1 = tmp.tile([128, 512], f32, tag="t1")
                        nc.vector.tensor_mul(t1[:], rps[:], csn_t[:, tsl])
                        t2 = tmp.tile([128, 512], f32, tag="t2")
                        nc.gpsimd.tensor_mul(t2[:], dst, csc_t[:, tsl])
                        nc.vector.tensor_add(dst, t1[:], t2[:])
                    else:
                        nc.vector.tensor_copy(vt[jc - 4][:, tsl], ps[:])
            # transpose this t-chunk's v^T into v [s, dh] + ones columns
            for b4 in range(4):
                i = 4 * tcc + b4
                vdst = v_sb[i]
                for vc in range(2):
                    pst = psT.tile([128, 128], f16, tag="pst")
                    nc.tensor.transpose(
                        pst[:], vt[vc][:, 128 * i:128 * (i + 1)], id_t[:])
                    ha = 2 * vc
                    dst3 = vdst[:, 65 * ha:65 * ha + 130].rearrange(
                        "p (h e) -> p h e", h=2, e=65)[:, :, 0:64]
                    src3 = pst[:].rearrange("p (h e) -> p h e", h=2, e=64)
                    nc.vector.tensor_copy(dst3, src3)
                nc.vector.tensor_copy(vdst[:, 64:260:65],
                                      oc_t[:, 0:1].broadcast_to([128, 4]))

    # ===== phase 3: causal attention ================================
    with tc.tile_pool(name="psS", bufs=3, space="PSUM") as psS, \
         tc.tile_pool(name="psO", bufs=1, space="PSUM") as psO:
        for p in range(NPAIR):
            qc = qk[2 * p]
            kch = qk[2 * p + 1]
            for j in range(TC512):
                tsl = slice(512 * j, 512 * (j + 1))
                po = [psO.tile([65, 512], f32, tag=f"o{hh}", name=f"po{hh}")
                      for hh in range(2)]
                ni = 4 * (j + 1)          # causal s-chunks for this strip
                for g in range(ni // 2):
                    pss = [psS.tile([128, 1024], f32, tag="s",
                                    name=f"pss{hh}") for hh in range(2)]
                    # scores: interleave the two heads' matmuls so their
                    # disjoint row-groups stream concurrently through the PE
                    for half in range(2):
                        ii = 2 * g + half
                        diag = ii >= 4 * j
                        r = ii - 4 * j if diag else 0
                        c0 = 512 * half + (128 * r if diag else 0)
                        qs = slice(512 * j + 128 * r, 512 * (j + 1)) \
                            if diag else tsl
                        for hh in range(2):
                            hsl = slice(64 * hh, 64 * (hh + 1))
                            nc.tensor.matmul(
                                pss[hh][:, c0:512 * half + 512],
                                kch[hsl, 128 * ii:128 * (ii + 1)],
                                qc[hsl, qs],
                                start=True, stop=not diag)
                        if diag:
                            w = PAT_W[r]
                            for hh in range(2):
                                nc.tensor.matmul(
                                    pss[hh][:, 512 * half:512 * half + w],
                                    id_t[:],
                                    pat_t[:, PAT_OFF[r]:PAT_OFF[r] + w],
                                    start=False, stop=True)
                    ptl = [ptp.tile([128, 1024], f16, tag=f"ptl{hh}",
                                    name=f"ptl{hh}") for hh in range(2)]
                    for hh in range(2):
                        nc.scalar.activation(ptl[hh][:], pss[hh][:], EXP,
                                             scale=0.125)
                    for half in range(2):
                        ii = 2 * g + half
                        diag = ii >= 4 * j
                        r = ii - 4 * j if diag else 0
                        c0 = 128 * r if diag else 0
                        for hh in range(2):
                            h = 2 * p + hh
                            nc.tensor.matmul(
                                po[hh][:, c0:512],
                                v_sb[ii][:, 65 * h:65 * h + 65],
                                ptl[hh][:, 512 * half + c0:512 * half + 512],
                                start=(ii == 0), stop=(ii == ni - 1))
                for hh in range(2):
                    sr = small.tile([1, 512], f32, tag="bsb")
                    nc.vector.tensor_copy(sr[:], po[hh][64:65, :])
                    rc = small.tile([1, 512], f32, tag="rc")
                    nc.vector.reciprocal_approx_fast(rc[:], sr[:])
                    bsb = small.tile([64, 512], f32, tag="bsb")
                    nc.gpsimd.partition_broadcast(bsb[:], rc[0:1, :])
                    nc.vector.tensor_mul(
                        attn[p][64 * hh:64 * (hh + 1), tsl],
                        po[hh][0:64, :], bsb[:])

    # ===== phase 4: output projection (partial) ======================
    with tc.tile_pool(name="psP", bufs=3, space="PSUM") as psP:
        n = 0
        for oc in range(D // 128):
            for tj in range(TC512):
                tsl = slice(512 * tj, 512 * (tj + 1))
                pp = psP.tile([128, 512], f32, tag="pp")
                for kc in range(2):
                    nc.tensor.matmul(
                        pp[:], wp_t[kc][:, 128 * oc:128 * (oc + 1)],
                        attn[kc][:, tsl],
                        start=(kc == 0), stop=(kc == 1))
                ob = stage.tile([128, 512], f32, tag="ob")
                nc.vector.tensor_copy(ob[:], pp[:])
                nc.sync.dma_start(d["o"][128 * oc:128 * (oc + 1), tsl], ob[:])
                n += 1


def _build_module():
    nc = bacc.Bacc("TRN2", target_bir_lowering=False, debug=False,
                   enable_asserts=False)
    d = {
        "xT": nc.dram_tensor("xT", [D, T], f16, kind="ExternalInput").ap(),
        "wqk": nc.dram_tensor("wqk", [D, 512], f16, kind="ExternalInput").ap(),
        "wv": nc.dram_tensor("wv", [D, 256], f16, kind="ExternalInput").ap(),
        "wp": nc.dram_tensor("wp", [256, D], f16, kind="ExternalInput").ap(),
        "csc": nc.dram_tensor("csc", [128, T], f16, kind="ExternalInput").ap(),
        "csn": nc.dram_tensor("csn", [128, T], f16, kind="ExternalInput").ap(),
        "rT": nc.dram_tensor("rT", [128, 128], f16, kind="ExternalInput").ap(),
        "ident": nc.dram_tensor("ident", [128, 128], f16,
                                kind="ExternalInput").ap(),
        "ones1": nc.dram_tensor("ones1", [1, 64], f16,
                                kind="ExternalInput").ap(),
        "onesc": nc.dram_tensor("onesc", [128, 1], f16,
                                kind="ExternalInput").ap(),
        "pat": nc.dram_tensor("pat", [128, 1280], f16,
                              kind="ExternalInput").ap(),
        "o": nc.dram_tensor("o", [D, T], f32, kind="ExternalOutput").ap(),
    }
    with tile.TileContext(nc) as tc:
        with ExitStack() as ctx, \
             nc.allow_low_precision("fp32r PE operands are rounded by design"):
            _emit(nc, tc, d, ctx)
    nc.compile()
    return nc


def _get_module():
    if "nc" not in _CACHE:
        _CACHE["nc"] = _build_module()
    return _CACHE["nc"]


def _canonical(attn_mask, key_padding_mask):
    if attn_mask.shape != (1, 1, T, T) or key_padding_mask.shape != (B, T):
        return False
    if not key_padding_mask.all():
        return False
    m = np.asarray(attn_mask[0, 0], dtype=np.float32)
    causal = np.triu(np.full((T, T), -1e9, dtype=np.float32), k=1)
    return np.array_equal(m, causal)


def _reference_fallback(x, attn_mask, key_padding_mask, Wqkv, Wproj):
    x = np.asarray(x, np.float32)
    qkv = x @ np.asarray(Wqkv, np.float32).T
    q, k, v = qkv[..., :D], qkv[..., D:2 * D], qkv[..., 2 * D:]

    def split(t):
        return t.reshape(B, -1, H, DH).transpose(0, 2, 1, 3)

    def rope(xx):
        inv = 1.0 / (10000.0 ** (np.arange(0, DH, 2, dtype=np.float32) / DH))
        fr = np.arange(T, dtype=np.float32)[:, None] * inv[None, :]
        emb = np.concatenate([fr, fr], axis=-1)
        cos, sin = np.cos(emb)[None, None], np.sin(emb)[None, None]
        x1, x2 = xx[..., ::2], xx[..., 1::2]
        rh = np.stack((-x2, x1), axis=-1).reshape(xx.shape)
        return xx * cos + rh * sin

    q, k, v = split(q), split(k), split(v)
    q, k = rope(q), rope(k)
    s = np.einsum("bhtd,bhsd->bhts", q, k) / np.sqrt(np.float32(DH))
    s = s + np.asarray(attn_mask, np.float32)
    s = np.where(np.asarray(key_padding_mask)[:, None, None, :], s, -1e9)
    s = s - s.max(axis=-1, keepdims=True)
    e = np.exp(s)
    a = e / e.sum(axis=-1, keepdims=True)
    out = np.einsum("bhts,bhsd->bhtd", a, v)
    out = out.transpose(0, 2, 1, 3).reshape(B, T, D)
    return out @ np.asarray(Wproj, np.float32).T


def _make_in_maps(x, Wqkv, Wproj):
    csc, csn = _rope_tables()
    rT = _rot_matrix().astype(np.float16)
    pat = _diag_patterns()
    ident = np.eye(128, dtype=np.float16)
    ones1 = np.ones((1, 64), dtype=np.float16)
    onesc = np.ones((128, 1), dtype=np.float16)

    Wq = np.asarray(Wqkv[:D], np.float32).reshape(H, DH, D)
    Wk = np.asarray(Wqkv[D:2 * D], np.float32).reshape(H, DH, D)
    Wv = np.asarray(Wqkv[2 * D:], np.float32).reshape(H, DH, D)
    WpT = np.ascontiguousarray(np.asarray(Wproj, np.float32).T)  # [din, dout]

    xT = [np.ascontiguousarray(np.asarray(x[b], np.float32).T.astype(np.float16))
          for b in range(B)]

    in_maps = []
    for c in range(NCORES):
        b, g = divmod(c, GROUPS)
        hs = [HPC * g + hl for hl in range(HPC)]  # global head ids
        cols = []
        for pp in range(NPAIR):
            h0, h1 = hs[2 * pp], hs[2 * pp + 1]
            cols.append(np.concatenate([Wq[h0], Wq[h1]], axis=0))  # [128, D]
            cols.append(np.concatenate([Wk[h0], Wk[h1]], axis=0))
        wqk = np.ascontiguousarray(
            np.concatenate(cols, axis=0).T.astype(np.float16))     # [D, 512]
        wv = np.ascontiguousarray(
            np.concatenate([Wv[h] for h in hs], axis=0).T.astype(np.float16))
        wp = np.ascontiguousarray(
            WpT[256 * g:256 * (g + 1), :].astype(np.float16))  # [256, D]
        in_maps.append({
            "xT": xT[b], "wqk": wqk, "wv": wv, "wp": wp,
            "csc": csc, "csn": csn, "rT": rT, "ident": ident,
            "ones1": ones1, "onesc": onesc, "pat": pat,
        })
    return in_maps


def _in_maps_for_trace(inputs):
    return _make_in_maps(np.asarray(inputs["x"]), np.asarray(inputs["Wqkv"]),
                         np.asarray(inputs["Wproj"]))


def kernel(x, attn_mask, key_padding_mask, Wqkv, Wproj):
    x = np.asarray(x)
    attn_mask = np.asarray(attn_mask)
    key_padding_mask = np.asarray(key_padding_mask)
    Wqkv = np.asarray(Wqkv)
    Wproj = np.asarray(Wproj)

    if not _canonical(attn_mask, key_padding_mask):
        return _reference_fallback(x, attn_mask, key_padding_mask, Wqkv, Wproj)

    nc = _get_module()
    in_maps = _make_in_maps(x, Wqkv, Wproj)
    res = bass_utils.run_bass_kernel_spmd(nc, in_maps,
                                          core_ids=list(range(NCORES)))
    out = np.empty((B, T, D), dtype=np.float32)
    for b in range(B):
        acc = res.results[4 * b]["o"].astype(np.float32).copy()
        for g in range(1, GROUPS):
            acc += res.results[4 * b + g]["o"]
        out[b] = acc.T
    return out


# revision 24
# speedup vs baseline: 1.0291x; 1.0291x over previous
"""Trainium2 Bass kernel for nn_MultiHeadAttention_85375359909998.

Causal MHA with (non-standard interleaved) RoPE, fp32 in/out.
  B=2, T=2048, D=1024, H=16, DH=64.

Sharding over 8 NeuronCores: data-parallel over batch (2) x tensor-parallel
over head groups (16 heads -> 4 groups of 4). Each core computes its batch's
QKV projection for its 4 heads, RoPE, causal attention, and a partial output
projection; the host sums the 4 partial projections per batch (the
"all-reduce") and concatenates batches.

Device-side layout notes (per core, heads grouped in pairs):
  - everything that feeds the PE is float32r (TF32-like, full-rate matmul)
  - q/k are produced *transposed* ([dh, t]) directly by the projection
    (host passes x^T and W^T); RoPE rotate-half is a 128x128 block-diagonal
    permutation matrix applied on the PE, combined with cos/sin on the DVE
  - scores are computed transposed (S^T[s, t]) so the A@V matmul can use
    P^T tiles as the moving operand with V ([s, dh]) stationary; V gets an
    appended ones-column so row-sums (softmax denominators) fall out of the
    same matmul; causal masking = skipping upper tiles + PE-adding a
    precomputed -inf pattern to the 4 diagonal blocks of each row strip
  - softmax normalization: reciprocal of the sums row, broadcast across
    partitions with a K=1 ones outer-product on the PE, multiplied on DVE.
"""

import sys
from contextlib import ExitStack

import numpy as np

try:
    import concourse.bass as bass  # noqa: F401
except ImportError:  # pragma: no cover
    sys.path.insert(0, "/opt/trn_rl_repo")
    import concourse.bass as bass  # noqa: F401

import concourse.tile as tile
from concourse import bacc, mybir
from concourse import bass_utils

B, T, D, H, DH = 2, 2048, 1024, 16, 64
NCORES = 8
GROUPS = 4          # head groups (tensor-parallel dimension)
HPC = H // GROUPS   # 4 heads per core
NPAIR = HPC // 2    # head pairs per core
TC512 = T // 512    # 4
SC128 = T // 128    # 16
KC = D // 128       # 8 contraction chunks for the projections
NEG8 = np.float32(-4.0e4)  # pre-scale additive mask: exp(NEG8/8) == 0, fits fp16

f32 = mybir.dt.float32
f32r = mybir.dt.float32r
f16 = mybir.dt.float16
EXP = mybir.ActivationFunctionType.Exp
COPY = mybir.ActivationFunctionType.Copy

PAT_OFF = [0, 128, 384, 768]
PAT_W = [128, 256, 384, 512]

_CACHE = {}


def _rope_tables():
    """cos/sin tables, transposed & stacked for the [2*64, t] chunk layout."""
    inv = 1.0 / (10000.0 ** (np.arange(0, DH, 2, dtype=np.float64) / DH))  # 32
    t = np.arange(T, dtype=np.float64)
    freqs = t[:, None] * inv[None, :]                 # [T, 32]
    emb = np.concatenate([freqs, freqs], axis=-1)     # [T, 64]
    cos = np.cos(emb).astype(np.float32).T            # [64, T]
    sin = np.sin(emb).astype(np.float32).T
    csc = np.concatenate([cos, cos], axis=0)          # [128, T]
    csn = np.concatenate([sin, sin], axis=0)
    return (np.ascontiguousarray(csc.astype(np.float16)),
            np.ascontiguousarray(csn.astype(np.float16)))


def _rot_matrix():
    """R.T for rotate_half: (R@v)[2i] = -v[2i+1], (R@v)[2i+1] = v[2i]."""
    R = np.zeros((DH, DH), dtype=np.float32)
    for i in range(DH // 2):
        R[2 * i, 2 * i + 1] = -1.0
        R[2 * i + 1, 2 * i] = 1.0
    R128 = np.zeros((128, 128), dtype=np.float32)
    R128[:DH, :DH] = R
    R128[DH:, DH:] = R
    return np.ascontiguousarray(R128.T)


def _diag_patterns():
    """Concatenated diagonal-block causal masks, widths 128/256/384/512."""
    blocks = []
    for r in range(4):
        w = 128 * (r + 1)
        sp = np.arange(128)[:, None]
        tp = np.arange(w)[None, :]
        blocks.append(np.where(tp < 128 * r + sp, NEG8, np.float32(0.0)))
    return np.ascontiguousarray(
        np.concatenate(blocks, axis=1).astype(np.float16))


def _emit(nc, tc, d, ctx):
    const = ctx.enter_context(tc.tile_pool(name="const", bufs=1))
    qkp = ctx.enter_context(tc.tile_pool(name="qk", bufs=1))
    vtp = ctx.enter_context(tc.tile_pool(name="vt", bufs=1))
    vp = ctx.enter_context(tc.tile_pool(name="v", bufs=1))
    att = ctx.enter_context(tc.tile_pool(name="att", bufs=1))
    ptp = ctx.enter_context(tc.tile_pool(name="pt", bufs=3))
    tmp = ctx.enter_context(tc.tile_pool(name="tmp", bufs=3))
    small = ctx.enter_context(tc.tile_pool(name="small", bufs=2))
    stage = ctx.enter_context(tc.tile_pool(name="stage", bufs=4))

    # ---- constants ----
    wqk_t, wv_t = [], []
    for kc in range(KC):
        w1 = const.tile([128, 512], f16, tag=f"wqk{kc}")
        nc.scalar.dma_start(w1[:], d["wqk"][128 * kc:128 * (kc + 1), :])
        wqk_t.append(w1)
        w2 = const.tile([128, 256], f16, tag=f"wv{kc}")
        nc.scalar.dma_start(w2[:], d["wv"][128 * kc:128 * (kc + 1), :])
        wv_t.append(w2)
    rT_t = const.tile([128, 128], f16, tag="rT")
    nc.scalar.dma_start(rT_t[:], d["rT"][:])
    csc_t = const.tile([128, T], f16, tag="csc")
    csn_t = const.tile([128, T], f16, tag="csn")
    id_t = const.tile([128, 128], f16, tag="ident")
    oc_t = const.tile([128, 1], f16, tag="onesc")
    pat_t = const.tile([128, 1280], f16, tag="pat")
    wp_t = [const.tile([128, D], f16, tag=f"wp{kc}", name=f"wp{kc}")
            for kc in range(2)]

    # ---- persistent activations ----
    xT_t = []
    for kc in range(KC):
        xt_ = const.tile([128, T], f16, tag=f"xt{kc}", name=f"xt{kc}")
        dma_eng = nc.sync if kc % 2 == 0 else nc.scalar
        dma_eng.dma_start(xt_[:], d["xT"][128 * kc:128 * (kc + 1), :])
        xT_t.append(xt_)

    qk = [qkp.tile([128, T], f16, tag=f"qk{j}", name=f"qk{j}")
          for j in range(4)]
    vt = [vtp.tile([128, T], f16, tag=f"vt{j}", name=f"vt{j}")
          for j in range(2)]
    v_sb = [vp.tile([128, 4 * 65], f16, tag=f"v{i}", name=f"v{i}")
            for i in range(SC128)]
    attn = [att.tile([128, T], f16, tag=f"at{p}", name=f"at{p}")
            for p in range(NPAIR)]

    # ===== phase 1: qkv projections (transposed) + RoPE + V transpose ==
    with tc.tile_pool(name="psA", bufs=4, space="PSUM") as psA, \
         tc.tile_pool(name="psR", bufs=2, space="PSUM") as psR, \
         tc.tile_pool(name="psT", bufs=2, space="PSUM") as psT:
        for tcc in range(TC512):
            tsl = slice(512 * tcc, 512 * (tcc + 1))
            xst = [xT_t[kc][:, tsl] for kc in range(KC)]
            if tcc == 0:
                # lower-priority constants: needed only from the RoPE /
                # transpose / attention / proj stages onward
                nc.gpsimd.dma_start(csc_t[:], d["csc"][:])
                nc.gpsimd.dma_start(csn_t[:], d["csn"][:])
                nc.gpsimd.dma_start(id_t[:], d["ident"][:])
                nc.gpsimd.dma_start(oc_t[:], d["onesc"][:])
            if tcc == 1:
                nc.gpsimd.dma_start(pat_t[:], d["pat"][:])
                for kc2 in range(2):
                    nc.gpsimd.dma_start(
                        wp_t[kc2][:], d["wp"][128 * kc2:128 * (kc2 + 1), :])
            for jp in range(3):
                pss2 = [psA.tile([128, 512], f32, tag="ps", name=f"ps{jc}")
                        for jc in (2 * jp, 2 * jp + 1)]
                for kc in range(KC):
                    for u in range(2):
                        jc = 2 * jp + u
                        lhsT = (wqk_t[kc][:, 128 * jc:128 * (jc + 1)]
                                if jc < 4 else
                                wv_t[kc][:, 128 * (jc - 4):128 * (jc - 3)])
                        nc.tensor.matmul(pss2[u][:], lhsT, xst[kc],
                                         start=(kc == 0), stop=(kc == KC - 1))
                for u in range(2):
                    jc = 2 * jp + u
                    ps = pss2[u]
                    if jc < 4:
                        dst = qk[jc][:, tsl]
                        nc.scalar.copy(dst, ps[:])
                        rps = psR.tile([128, 512], f32, tag="rps")
                        nc.tensor.matmul(rps[:], rT_t[:], dst,
                                         start=True, stop=True)
                        t1 = tmp.tile([128, 512], f32, tag="t1")
                        nc.vector.tensor_mul(t1[:], rps[:], csn_t[:, tsl])
                        t2 = tmp.tile([128, 512], f32, tag="t2")
                        nc.gpsimd.tensor_mul(t2[:], dst, csc_t[:, tsl])
                        nc.vector.tensor_add(dst, t1[:], t2[:])
                    else:
                        nc.scalar.copy(vt[jc - 4][:, tsl], ps[:])
            # transpose this t-chunk's v^T into v [s, dh] + ones columns
            for b4 in range(4):
                i = 4 * tcc + b4
                vdst = v_sb[i]
                for vc in range(2):
                    pst = psT.tile([128, 128], f16, tag="pst")
                    nc.tensor.transpose(
                        pst[:], vt[vc][:, 128 * i:128 * (i + 1)], id_t[:])
                    ha = 2 * vc
                    dst3 = vdst[:, 65 * ha:65 * ha + 130].rearrange(
                        "p (h e) -> p h e", h=2, e=65)[:, :, 0:64]
                    src3 = pst[:].rearrange("p (h e) -> p h e", h=2, e=64)
                    nc.vector.tensor_copy(dst3, src3)
                nc.vector.tensor_copy(vdst[:, 64:260:65],
                                      oc_t[:, 0:1].broadcast_to([128, 4]))

    # ===== phase 3: causal attention ================================
    with tc.tile_pool(name="psS", bufs=3, space="PSUM") as psS, \
         tc.tile_pool(name="psO", bufs=1, space="PSUM") as psO:
        for p in range(NPAIR):
            qc = qk[2 * p]
            kch = qk[2 * p + 1]
            for j in range(TC512):
                tsl = slice(512 * j, 512 * (j + 1))
                po = [psO.tile([65, 512], f32, tag=f"o{hh}", name=f"po{hh}")
                      for hh in range(2)]
                ni = 4 * (j + 1)          # causal s-chunks for this strip
                for g in range(ni // 2):
                    pss = [psS.tile([128, 1024], f32, tag="s",
                                    name=f"pss{hh}") for hh in range(2)]
                    # scores: interleave the two heads' matmuls so their
                    # disjoint row-groups stream concurrently through the PE
                    for half in range(2):
                        ii = 2 * g + half
                        diag = ii >= 4 * j
                        r = ii - 4 * j if diag else 0
                        c0 = 512 * half + (128 * r if diag else 0)
                        qs = slice(512 * j + 128 * r, 512 * (j + 1)) \
                            if diag else tsl
                        for hh in range(2):
                            hsl = slice(64 * hh, 64 * (hh + 1))
                            nc.tensor.matmul(
                                pss[hh][:, c0:512 * half + 512],
                                kch[hsl, 128 * ii:128 * (ii + 1)],
                                qc[hsl, qs],
                                start=True, stop=not diag)
                        if diag:
                            w = PAT_W[r]
                            for hh in range(2):
                                nc.tensor.matmul(
                                    pss[hh][:, 512 * half:512 * half + w],
                                    id_t[:],
                                    pat_t[:, PAT_OFF[r]:PAT_OFF[r] + w],
                                    start=False, stop=True)
                    ptl = [ptp.tile([128, 1024], f16, tag=f"ptl{hh}",
                                    name=f"ptl{hh}") for hh in range(2)]
                    for hh in range(2):
                        nc.scalar.activation(ptl[hh][:], pss[hh][:], EXP,
                                             scale=0.125)
                    for half in range(2):
                        ii = 2 * g + half
                        diag = ii >= 4 * j
                        r = ii - 4 * j if diag else 0
                        c0 = 128 * r if diag else 0
                        for hh in range(2):
                            h = 2 * p + hh
                            nc.tensor.matmul(
                                po[hh][:, c0:512],
                                v_sb[ii][:, 65 * h:65 * h + 65],
                                ptl[hh][:, 512 * half + c0:512 * half + 512],
                                start=(ii == 0), stop=(ii == ni - 1))
                for hh in range(2):
                    sr = small.tile([1, 512], f32, tag="bsb")
                    nc.vector.tensor_copy(sr[:], po[hh][64:65, :])
                    rc = small.tile([1, 512], f32, tag="rc")
                    nc.vector.reciprocal_approx_fast(rc[:], sr[:])
                    bsb = small.tile([64, 512], f32, tag="bsb")
                    nc.gpsimd.partition_broadcast(bsb[:], rc[0:1, :])
                    nc.vector.tensor_mul(
                        attn[p][64 * hh:64 * (hh + 1), tsl],
                        po[hh][0:64, :], bsb[:])

    # ===== phase 4: output projection (partial) ======================
    with tc.tile_pool(name="psP", bufs=3, space="PSUM") as psP:
        n = 0
        for oc in range(D // 128):
            for tj in range(TC512):
                tsl = slice(512 * tj, 512 * (tj + 1))
                pp = psP.tile([128, 512], f32, tag="pp")
                for kc in range(2):
                    nc.tensor.matmul(
                        pp[:], wp_t[kc][:, 128 * oc:128 * (oc + 1)],
                        attn[kc][:, tsl],
                        start=(kc == 0), stop=(kc == 1))
                ob = stage.tile([128, 512], f32, tag="ob")
                if n % 2 == 0:
                    nc.vector.tensor_copy(ob[:], pp[:])
                else:
                    nc.scalar.activation(ob[:], pp[:], COPY)
                nc.sync.dma_start(d["o"][128 * oc:128 * (oc + 1), tsl], ob[:])
                n += 1


def _build_module():
    nc = bacc.Bacc("TRN2", target_bir_lowering=False, debug=False,
                   enable_asserts=False)
    d = {
        "xT": nc.dram_tensor("xT", [D, T], f16, kind="ExternalInput").ap(),
        "wqk": nc.dram_tensor("wqk", [D, 512], f16, kind="ExternalInput").ap(),
        "wv": nc.dram_tensor("wv", [D, 256], f16, kind="ExternalInput").ap(),
        "wp": nc.dram_tensor("wp", [256, D], f16, kind="ExternalInput").ap(),
        "csc": nc.dram_tensor("csc", [128, T], f16, kind="ExternalInput").ap(),
        "csn": nc.dram_tensor("csn", [128, T], f16, kind="ExternalInput").ap(),
        "rT": nc.dram_tensor("rT", [128, 128], f16, kind="ExternalInput").ap(),
        "ident": nc.dram_tensor("ident", [128, 128], f16,
                                kind="ExternalInput").ap(),
        "ones1": nc.dram_tensor("ones1", [1, 64], f16,
                                kind="ExternalInput").ap(),
        "onesc": nc.dram_tensor("onesc", [128, 1], f16,
                                kind="ExternalInput").ap(),
        "pat": nc.dram_tensor("pat", [128, 1280], f16,
                              kind="ExternalInput").ap(),
        "o": nc.dram_tensor("o", [D, T], f32, kind="ExternalOutput").ap(),
    }
    with tile.TileContext(nc) as tc:
        with ExitStack() as ctx, \
             nc.allow_low_precision("fp32r PE operands are rounded by design"):
            _emit(nc, tc, d, ctx)
    nc.compile()
    return nc


def _get_module():
    if "nc" not in _CACHE:
        _CACHE["nc"] = _build_module()
    return _CACHE["nc"]


def _canonical(attn_mask, key_padding_mask):
    if attn_mask.shape != (1, 1, T, T) or key_padding_mask.shape != (B, T):
        return False
    if not key_padding_mask.all():
        return False
    m = np.asarray(attn_mask[0, 0], dtype=np.float32)
    causal = np.triu(np.full((T, T), -1e9, dtype=np.float32), k=1)
    return np.array_equal(m, causal)


def _reference_fallback(x, attn_mask, key_padding_mask, Wqkv, Wproj):
    x = np.asarray(x, np.float32)
    qkv = x @ np.asarray(Wqkv, np.float32).T
    q, k, v = qkv[..., :D], qkv[..., D:2 * D], qkv[..., 2 * D:]

    def split(t):
        return t.reshape(B, -1, H, DH).transpose(0, 2, 1, 3)

    def rope(xx):
        inv = 1.0 / (10000.0 ** (np.arange(0, DH, 2, dtype=np.float32) / DH))
        fr = np.arange(T, dtype=np.float32)[:, None] * inv[None, :]
        emb = np.concatenate([fr, fr], axis=-1)
        cos, sin = np.cos(emb)[None, None], np.sin(emb)[None, None]
        x1, x2 = xx[..., ::2], xx[..., 1::2]
        rh = np.stack((-x2, x1), axis=-1).reshape(xx.shape)
        return xx * cos + rh * sin

    q, k, v = split(q), split(k), split(v)
    q, k = rope(q), rope(k)
    s = np.einsum("bhtd,bhsd->bhts", q, k) / np.sqrt(np.float32(DH))
    s = s + np.asarray(attn_mask, np.float32)
    s = np.where(np.asarray(key_padding_mask)[:, None, None, :], s, -1e9)
    s = s - s.max(axis=-1, keepdims=True)
    e = np.exp(s)
    a = e / e.sum(axis=-1, keepdims=True)
    out = np.einsum("bhts,bhsd->bhtd", a, v)
    out = out.transpose(0, 2, 1, 3).reshape(B, T, D)
    return out @ np.asarray(Wproj, np.float32).T


def _make_in_maps(x, Wqkv, Wproj):
    csc, csn = _rope_tables()
    rT = _rot_matrix().astype(np.float16)
    pat = _diag_patterns()
    ident = np.eye(128, dtype=np.float16)
    ones1 = np.ones((1, 64), dtype=np.float16)
    onesc = np.ones((128, 1), dtype=np.float16)

    Wq = np.asarray(Wqkv[:D], np.float32).reshape(H, DH, D)
    Wk = np.asarray(Wqkv[D:2 * D], np.float32).reshape(H, DH, D)
    Wv = np.asarray(Wqkv[2 * D:], np.float32).reshape(H, DH, D)
    WpT = np.ascontiguousarray(np.asarray(Wproj, np.float32).T)  # [din, dout]

    xT = [np.ascontiguousarray(np.asarray(x[b], np.float32).T.astype(np.float16))
          for b in range(B)]

    in_maps = []
    for c in range(NCORES):
        b, g = divmod(c, GROUPS)
        hs = [HPC * g + hl for hl in range(HPC)]  # global head ids
        cols = []
        for pp in range(NPAIR):
            h0, h1 = hs[2 * pp], hs[2 * pp + 1]
            cols.append(np.concatenate([Wq[h0], Wq[h1]], axis=0))  # [128, D]
            cols.append(np.concatenate([Wk[h0], Wk[h1]], axis=0))
        wqk = np.ascontiguousarray(
            np.concatenate(cols, axis=0).T.astype(np.float16))     # [D, 512]
        wv = np.ascontiguousarray(
            np.concatenate([Wv[h] for h in hs], axis=0).T.astype(np.float16))
        wp = np.ascontiguousarray(
            WpT[256 * g:256 * (g + 1), :].astype(np.float16))  # [256, D]
        in_maps.append({
            "xT": xT[b], "wqk": wqk, "wv": wv, "wp": wp,
            "csc": csc, "csn": csn, "rT": rT, "ident": ident,
            "ones1": ones1, "onesc": onesc, "pat": pat,
        })
    return in_maps


def _in_maps_for_trace(inputs):
    return _make_in_maps(np.asarray(inputs["x"]), np.asarray(inputs["Wqkv"]),
                         np.asarray(inputs["Wproj"]))


def kernel(x, attn_mask, key_padding_mask, Wqkv, Wproj):
    x = np.asarray(x)
    attn_mask = np.asarray(attn_mask)
    key_padding_mask = np.asarray(key_padding_mask)
    Wqkv = np.asarray(Wqkv)
    Wproj = np.asarray(Wproj)

    if not _canonical(attn_mask, key_padding_mask):
        return _reference_fallback(x, attn_mask, key_padding_mask, Wqkv, Wproj)

    nc = _get_module()
    in_maps = _make_in_maps(x, Wqkv, Wproj)
    res = bass_utils.run_bass_kernel_spmd(nc, in_maps,
                                          core_ids=list(range(NCORES)))
    out = np.empty((B, T, D), dtype=np.float32)
    for b in range(B):
        acc = res.results[4 * b]["o"].astype(np.float32).copy()
        for g in range(1, GROUPS):
            acc += res.results[4 * b + g]["o"]
        out[b] = acc.T
    return out


# revision 25
# speedup vs baseline: 1.0548x; 1.0249x over previous
"""Trainium2 Bass kernel for nn_MultiHeadAttention_85375359909998.

Causal MHA with (non-standard interleaved) RoPE, fp32 in/out.
  B=2, T=2048, D=1024, H=16, DH=64.

Sharding over 8 NeuronCores: data-parallel over batch (2) x tensor-parallel
over head groups (16 heads -> 4 groups of 4). Each core computes its batch's
QKV projection for its 4 heads, RoPE, causal attention, and a partial output
projection; the host sums the 4 partial projections per batch (the
"all-reduce") and concatenates batches.

Device-side layout notes (per core, heads grouped in pairs):
  - PE operands are fp16 (1 cycle/column streaming + fast weight loads);
    accumulation stays fp32 in PSUM. Measured end-to-end error vs the
    fp32 reference: max-rel ~5e-4, resid_var ~6e-7.
  - q/k are produced *transposed* ([dh, t]) directly by the projection
    (host passes x^T and W^T); RoPE rotate-half is a 128x128 block-diagonal
    permutation matrix applied on the PE, combined with cos/sin muls split
    across DVE and GpSimd; evictions ride the Scalar engine
  - scores are computed transposed (S^T[s, t]) so the A@V matmul can use
    P^T tiles as the moving operand with V ([s, dh]) stationary; V gets an
    appended ones-column so row-sums (softmax denominators) fall out of the
    same matmul; causal masking = skipping upper tiles/columns + an
    accumulating identity-matmul that adds a precomputed -inf pattern to
    the diagonal blocks
  - softmax normalization: DVE fast-reciprocal of the sums row, GpSimd
    partition-broadcast, DVE multiply.
"""

import sys
from contextlib import ExitStack

import numpy as np

try:
    import concourse.bass as bass  # noqa: F401
except ImportError:  # pragma: no cover
    sys.path.insert(0, "/opt/trn_rl_repo")
    import concourse.bass as bass  # noqa: F401

import concourse.tile as tile
from concourse import bacc, mybir
from concourse import bass_utils

B, T, D, H, DH = 2, 2048, 1024, 16, 64
NCORES = 8
GROUPS = 4          # head groups (tensor-parallel dimension)
HPC = H // GROUPS   # 4 heads per core
NPAIR = HPC // 2    # head pairs per core
TC512 = T // 512    # 4
SC128 = T // 128    # 16
KC = D // 128       # 8 contraction chunks for the projections
NEG8 = np.float32(-4.0e4)  # pre-scale additive mask: exp(NEG8/8) == 0, fits fp16

f32 = mybir.dt.float32
f32r = mybir.dt.float32r
f16 = mybir.dt.float16
EXP = mybir.ActivationFunctionType.Exp
COPY = mybir.ActivationFunctionType.Copy

PAT_OFF = [0, 128, 384, 768]
PAT_W = [128, 256, 384, 512]

_CACHE = {}


def _rope_tables():
    """cos/sin tables, transposed & stacked for the [2*64, t] chunk layout."""
    inv = 1.0 / (10000.0 ** (np.arange(0, DH, 2, dtype=np.float64) / DH))  # 32
    t = np.arange(T, dtype=np.float64)
    freqs = t[:, None] * inv[None, :]                 # [T, 32]
    emb = np.concatenate([freqs, freqs], axis=-1)     # [T, 64]
    cos = np.cos(emb).astype(np.float32).T            # [64, T]
    sin = np.sin(emb).astype(np.float32).T
    csc = np.concatenate([cos, cos], axis=0)          # [128, T]
    csn = np.concatenate([sin, sin], axis=0)
    return (np.ascontiguousarray(csc.astype(np.float16)),
            np.ascontiguousarray(csn.astype(np.float16)))


def _rot_matrix():
    """R.T for rotate_half: (R@v)[2i] = -v[2i+1], (R@v)[2i+1] = v[2i]."""
    R = np.zeros((DH, DH), dtype=np.float32)
    for i in range(DH // 2):
        R[2 * i, 2 * i + 1] = -1.0
        R[2 * i + 1, 2 * i] = 1.0
    R128 = np.zeros((128, 128), dtype=np.float32)
    R128[:DH, :DH] = R
    R128[DH:, DH:] = R
    return np.ascontiguousarray(R128.T)


def _diag_patterns():
    """Concatenated diagonal-block causal masks, widths 128/256/384/512."""
    blocks = []
    for r in range(4):
        w = 128 * (r + 1)
        sp = np.arange(128)[:, None]
        tp = np.arange(w)[None, :]
        blocks.append(np.where(tp < 128 * r + sp, NEG8, np.float32(0.0)))
    return np.ascontiguousarray(
        np.concatenate(blocks, axis=1).astype(np.float16))


def _emit(nc, tc, d, ctx):
    const = ctx.enter_context(tc.tile_pool(name="const", bufs=1))
    qkp = ctx.enter_context(tc.tile_pool(name="qk", bufs=1))
    vtp = ctx.enter_context(tc.tile_pool(name="vt", bufs=1))
    vp = ctx.enter_context(tc.tile_pool(name="v", bufs=1))
    att = ctx.enter_context(tc.tile_pool(name="att", bufs=1))
    ptp = ctx.enter_context(tc.tile_pool(name="pt", bufs=3))
    tmp = ctx.enter_context(tc.tile_pool(name="tmp", bufs=3))
    small = ctx.enter_context(tc.tile_pool(name="small", bufs=2))
    stage = ctx.enter_context(tc.tile_pool(name="stage", bufs=4))

    # ---- constants ----
    wqk_t, wv_t = [], []
    for kc in range(KC):
        w1 = const.tile([128, 512], f16, tag=f"wqk{kc}")
        nc.scalar.dma_start(w1[:], d["wqk"][128 * kc:128 * (kc + 1), :])
        wqk_t.append(w1)
        w2 = const.tile([128, 256], f16, tag=f"wv{kc}")
        nc.scalar.dma_start(w2[:], d["wv"][128 * kc:128 * (kc + 1), :])
        wv_t.append(w2)
    rT_t = const.tile([128, 128], f16, tag="rT")
    nc.scalar.dma_start(rT_t[:], d["rT"][:])
    csc_t = const.tile([128, T], f16, tag="csc")
    csn_t = const.tile([128, T], f16, tag="csn")
    id_t = const.tile([128, 128], f16, tag="ident")
    oc_t = const.tile([128, 1], f16, tag="onesc")
    pat_t = const.tile([128, 1280], f16, tag="pat")
    wp_t = [const.tile([128, D], f16, tag=f"wp{kc}", name=f"wp{kc}")
            for kc in range(2)]

    # ---- persistent activations ----
    xT_t = []
    for kc in range(KC):
        xt_ = const.tile([128, T], f16, tag=f"xt{kc}", name=f"xt{kc}")
        dma_eng = nc.sync if kc % 2 == 0 else nc.scalar
        dma_eng.dma_start(xt_[:], d["xT"][128 * kc:128 * (kc + 1), :])
        xT_t.append(xt_)

    qk = [qkp.tile([128, T], f16, tag=f"qk{j}", name=f"qk{j}")
          for j in range(4)]
    vt = [vtp.tile([128, T], f16, tag=f"vt{j}", name=f"vt{j}")
          for j in range(2)]
    v_sb = [vp.tile([128, 4 * 65], f16, tag=f"v{i}", name=f"v{i}")
            for i in range(SC128)]
    attn = [att.tile([128, T], f16, tag=f"at{p}", name=f"at{p}")
            for p in range(NPAIR)]

    # ===== phase 1: qkv projections (transposed) + RoPE + V transpose ==
    with tc.tile_pool(name="psA", bufs=4, space="PSUM") as psA, \
         tc.tile_pool(name="psR", bufs=2, space="PSUM") as psR, \
         tc.tile_pool(name="psT", bufs=2, space="PSUM") as psT:
        for tcc in range(TC512):
            tsl = slice(512 * tcc, 512 * (tcc + 1))
            xst = [xT_t[kc][:, tsl] for kc in range(KC)]
            if tcc == 0:
                # lower-priority constants: needed only from the RoPE /
                # transpose / attention / proj stages onward
                nc.gpsimd.dma_start(csc_t[:], d["csc"][:])
                nc.gpsimd.dma_start(csn_t[:], d["csn"][:])
                nc.gpsimd.dma_start(id_t[:], d["ident"][:])
                nc.gpsimd.dma_start(oc_t[:], d["onesc"][:])
            if tcc == 1:
                nc.gpsimd.dma_start(pat_t[:], d["pat"][:])
                for kc2 in range(2):
                    nc.gpsimd.dma_start(
                        wp_t[kc2][:], d["wp"][128 * kc2:128 * (kc2 + 1), :])
            for jp in range(3):
                pss2 = [psA.tile([128, 512], f32, tag="ps", name=f"ps{jc}")
                        for jc in (2 * jp, 2 * jp + 1)]
                for kc in range(KC):
                    for u in range(2):
                        jc = 2 * jp + u
                        lhsT = (wqk_t[kc][:, 128 * jc:128 * (jc + 1)]
                                if jc < 4 else
                                wv_t[kc][:, 128 * (jc - 4):128 * (jc - 3)])
                        nc.tensor.matmul(pss2[u][:], lhsT, xst[kc],
                                         start=(kc == 0), stop=(kc == KC - 1))
                for u in range(2):
                    jc = 2 * jp + u
                    ps = pss2[u]
                    if jc < 4:
                        dst = qk[jc][:, tsl]
                        nc.scalar.copy(dst, ps[:])
                        rps = psR.tile([128, 512], f32, tag="rps")
                        nc.tensor.matmul(rps[:], rT_t[:], dst,
                                         start=True, stop=True)
                        t1 = tmp.tile([128, 512], f32, tag="t1")
                        nc.vector.tensor_mul(t1[:], rps[:], csn_t[:, tsl])
                        t2 = tmp.tile([128, 512], f32, tag="t2")
                        nc.gpsimd.tensor_mul(t2[:], dst, csc_t[:, tsl])
                        nc.vector.tensor_add(dst, t1[:], t2[:])
                    else:
                        nc.scalar.copy(vt[jc - 4][:, tsl], ps[:])
            # transpose this t-chunk's v^T into v [s, dh] + ones columns
            for b4 in range(4):
                i = 4 * tcc + b4
                vdst = v_sb[i]
                for vc in range(2):
                    pst = psT.tile([128, 128], f16, tag="pst")
                    nc.tensor.transpose(
                        pst[:], vt[vc][:, 128 * i:128 * (i + 1)], id_t[:])
                    ha = 2 * vc
                    dst3 = vdst[:, 65 * ha:65 * ha + 130].rearrange(
                        "p (h e) -> p h e", h=2, e=65)[:, :, 0:64]
                    src3 = pst[:].rearrange("p (h e) -> p h e", h=2, e=64)
                    nc.vector.tensor_copy(dst3, src3)
                nc.vector.tensor_copy(vdst[:, 64:260:65],
                                      oc_t[:, 0:1].broadcast_to([128, 4]))

    # ===== phase 3: causal attention ================================
    with tc.tile_pool(name="psS", bufs=3, space="PSUM") as psS, \
         tc.tile_pool(name="psO", bufs=1, space="PSUM") as psO:
        for p in range(NPAIR):
            qc = qk[2 * p]
            kch = qk[2 * p + 1]
            for j in range(TC512):
                tsl = slice(512 * j, 512 * (j + 1))
                po = [psO.tile([65, 512], f32, tag=f"o{hh}", name=f"po{hh}")
                      for hh in range(2)]
                ni = 4 * (j + 1)          # causal s-chunks for this strip
                for g in range(ni // 2):
                    pss = [psS.tile([128, 1024], f32, tag="s",
                                    name=f"pss{hh}") for hh in range(2)]
                    # scores: interleave the two heads' matmuls so their
                    # disjoint row-groups stream concurrently through the PE
                    for half in range(2):
                        ii = 2 * g + half
                        diag = ii >= 4 * j
                        r = ii - 4 * j if diag else 0
                        c0 = 512 * half + (128 * r if diag else 0)
                        qs = slice(512 * j + 128 * r, 512 * (j + 1)) \
                            if diag else tsl
                        for hh in range(2):
                            hsl = slice(64 * hh, 64 * (hh + 1))
                            nc.tensor.matmul(
                                pss[hh][:, c0:512 * half + 512],
                                kch[hsl, 128 * ii:128 * (ii + 1)],
                                qc[hsl, qs],
                                start=True, stop=not diag)
                        if diag:
                            w = PAT_W[r]
                            for hh in range(2):
                                nc.tensor.matmul(
                                    pss[hh][:, 512 * half:512 * half + w],
                                    id_t[:],
                                    pat_t[:, PAT_OFF[r]:PAT_OFF[r] + w],
                                    start=False, stop=True)
                    ptl = [ptp.tile([128, 1024], f16, tag=f"ptl{hh}",
                                    name=f"ptl{hh}") for hh in range(2)]
                    for hh in range(2):
                        nc.scalar.activation(ptl[hh][:], pss[hh][:], EXP,
                                             scale=0.125)
                    for half in range(2):
                        ii = 2 * g + half
                        diag = ii >= 4 * j
                        r = ii - 4 * j if diag else 0
                        c0 = 128 * r if diag else 0
                        for hh in range(2):
                            h = 2 * p + hh
                            nc.tensor.matmul(
                                po[hh][:, c0:512],
                                v_sb[ii][:, 65 * h:65 * h + 65],
                                ptl[hh][:, 512 * half + c0:512 * half + 512],
                                start=(ii == 0), stop=(ii == ni - 1))
                for hh in range(2):
                    sr = small.tile([1, 512], f32, tag="bsb")
                    nc.vector.tensor_copy(sr[:], po[hh][64:65, :])
                    rc = small.tile([1, 512], f32, tag="rc")
                    nc.vector.reciprocal_approx_fast(rc[:], sr[:])
                    bsb = small.tile([64, 512], f32, tag="bsb")
                    nc.gpsimd.partition_broadcast(bsb[:], rc[0:1, :])
                    nc.vector.tensor_mul(
                        attn[p][64 * hh:64 * (hh + 1), tsl],
                        po[hh][0:64, :], bsb[:])

    # ===== phase 4: output projection (partial) ======================
    with tc.tile_pool(name="psP", bufs=3, space="PSUM") as psP:
        n = 0
        for oc in range(D // 128):
            for tj in range(TC512):
                tsl = slice(512 * tj, 512 * (tj + 1))
                pp = psP.tile([128, 512], f32, tag="pp")
                for kc in range(2):
                    nc.tensor.matmul(
                        pp[:], wp_t[kc][:, 128 * oc:128 * (oc + 1)],
                        attn[kc][:, tsl],
                        start=(kc == 0), stop=(kc == 1))
                ob = stage.tile([128, 512], f32, tag="ob")
                if n % 2 == 0:
                    nc.vector.tensor_copy(ob[:], pp[:])
                else:
                    nc.scalar.activation(ob[:], pp[:], COPY)
                nc.sync.dma_start(d["o"][128 * oc:128 * (oc + 1), tsl], ob[:])
                n += 1


def _build_module():
    nc = bacc.Bacc("TRN2", target_bir_lowering=False, debug=False,
                   enable_asserts=False)
    d = {
        "xT": nc.dram_tensor("xT", [D, T], f16, kind="ExternalInput").ap(),
        "wqk": nc.dram_tensor("wqk", [D, 512], f16, kind="ExternalInput").ap(),
        "wv": nc.dram_tensor("wv", [D, 256], f16, kind="ExternalInput").ap(),
        "wp": nc.dram_tensor("wp", [256, D], f16, kind="ExternalInput").ap(),
        "csc": nc.dram_tensor("csc", [128, T], f16, kind="ExternalInput").ap(),
        "csn": nc.dram_tensor("csn", [128, T], f16, kind="ExternalInput").ap(),
        "rT": nc.dram_tensor("rT", [128, 128], f16, kind="ExternalInput").ap(),
        "ident": nc.dram_tensor("ident", [128, 128], f16,
                                kind="ExternalInput").ap(),
        "ones1": nc.dram_tensor("ones1", [1, 64], f16,
                                kind="ExternalInput").ap(),
        "onesc": nc.dram_tensor("onesc", [128, 1], f16,
                                kind="ExternalInput").ap(),
        "pat": nc.dram_tensor("pat", [128, 1280], f16,
                              kind="ExternalInput").ap(),
        "o": nc.dram_tensor("o", [D, T], f32, kind="ExternalOutput").ap(),
    }
    with tile.TileContext(nc) as tc:
        with ExitStack() as ctx, \
             nc.allow_low_precision("fp32r PE operands are rounded by design"):
            _emit(nc, tc, d, ctx)
    nc.compile()
    return nc


def _get_module():
    if "nc" not in _CACHE:
        _CACHE["nc"] = _build_module()
    return _CACHE["nc"]


def _canonical(attn_mask, key_padding_mask):
    if attn_mask.shape != (1, 1, T, T) or key_padding_mask.shape != (B, T):
        return False
    if not key_padding_mask.all():
        return False
    m = np.asarray(attn_mask[0, 0], dtype=np.float32)
    causal = np.triu(np.full((T, T), -1e9, dtype=np.float32), k=1)
    return np.array_equal(m, causal)


def _reference_fallback(x, attn_mask, key_padding_mask, Wqkv, Wproj):
    x = np.asarray(x, np.float32)
    qkv = x @ np.asarray(Wqkv, np.float32).T
    q, k, v = qkv[..., :D], qkv[..., D:2 * D], qkv[..., 2 * D:]

    def split(t):
        return t.reshape(B, -1, H, DH).transpose(0, 2, 1, 3)

    def rope(xx):
        inv = 1.0 / (10000.0 ** (np.arange(0, DH, 2, dtype=np.float32) / DH))
        fr = np.arange(T, dtype=np.float32)[:, None] * inv[None, :]
        emb = np.concatenate([fr, fr], axis=-1)
        cos, sin = np.cos(emb)[None, None], np.sin(emb)[None, None]
        x1, x2 = xx[..., ::2], xx[..., 1::2]
        rh = np.stack((-x2, x1), axis=-1).reshape(xx.shape)
        return xx * cos + rh * sin

    q, k, v = split(q), split(k), split(v)
    q, k = rope(q), rope(k)
    s = np.einsum("bhtd,bhsd->bhts", q, k) / np.sqrt(np.float32(DH))
    s = s + np.asarray(attn_mask, np.float32)
    s = np.where(np.asarray(key_padding_mask)[:, None, None, :], s, -1e9)
    s = s - s.max(axis=-1, keepdims=True)
    e = np.exp(s)
    a = e / e.sum(axis=-1, keepdims=True)
    out = np.einsum("bhts,bhsd->bhtd", a, v)
    out = out.transpose(0, 2, 1, 3).reshape(B, T, D)
    return out @ np.asarray(Wproj, np.float32).T


def _make_in_maps(x, Wqkv, Wproj):
    csc, csn = _rope_tables()
    rT = _rot_matrix().astype(np.float16)
    pat = _diag_patterns()
    ident = np.eye(128, dtype=np.float16)
    ones1 = np.ones((1, 64), dtype=np.float16)
    onesc = np.ones((128, 1), dtype=np.float16)

    Wq = np.asarray(Wqkv[:D], np.float32).reshape(H, DH, D)
    Wk = np.asarray(Wqkv[D:2 * D], np.float32).reshape(H, DH, D)
    Wv = np.asarray(Wqkv[2 * D:], np.float32).reshape(H, DH, D)
    WpT = np.ascontiguousarray(np.asarray(Wproj, np.float32).T)  # [din, dout]

    xT = [np.ascontiguousarray(np.asarray(x[b], np.float32).T.astype(np.float16))
          for b in range(B)]

    in_maps = []
    for c in range(NCORES):
        b, g = divmod(c, GROUPS)
        hs = [HPC * g + hl for hl in range(HPC)]  # global head ids
        cols = []
        for pp in range(NPAIR):
            h0, h1 = hs[2 * pp], hs[2 * pp + 1]
            cols.append(np.concatenate([Wq[h0], Wq[h1]], axis=0))  # [128, D]
            cols.append(np.concatenate([Wk[h0], Wk[h1]], axis=0))
        wqk = np.ascontiguousarray(
            np.concatenate(cols, axis=0).T.astype(np.float16))     # [D, 512]
        wv = np.ascontiguousarray(
            np.concatenate([Wv[h] for h in hs], axis=0).T.astype(np.float16))
        wp = np.ascontiguousarray(
            WpT[256 * g:256 * (g + 1), :].astype(np.float16))  # [256, D]
        in_maps.append({
            "xT": xT[b], "wqk": wqk, "wv": wv, "wp": wp,
            "csc": csc, "csn": csn, "rT": rT, "ident": ident,
            "ones1": ones1, "onesc": onesc, "pat": pat,
        })
    return in_maps


def _in_maps_for_trace(inputs):
    return _make_in_maps(np.asarray(inputs["x"]), np.asarray(inputs["Wqkv"]),
                         np.asarray(inputs["Wproj"]))


def kernel(x, attn_mask, key_padding_mask, Wqkv, Wproj):
    x = np.asarray(x)
    attn_mask = np.asarray(attn_mask)
    key_padding_mask = np.asarray(key_padding_mask)
    Wqkv = np.asarray(Wqkv)
    Wproj = np.asarray(Wproj)

    if not _canonical(attn_mask, key_padding_mask):
        return _reference_fallback(x, attn_mask, key_padding_mask, Wqkv, Wproj)

    nc = _get_module()
    in_maps = _make_in_maps(x, Wqkv, Wproj)
    res = bass_utils.run_bass_kernel_spmd(nc, in_maps,
                                          core_ids=list(range(NCORES)))
    out = np.empty((B, T, D), dtype=np.float32)
    for b in range(B):
        acc = res.results[4 * b]["o"].astype(np.float32).copy()
        for g in range(1, GROUPS):
            acc += res.results[4 * b + g]["o"]
        out[b] = acc.T
    return out


# revision 27
# speedup vs baseline: 1.0654x; 1.0101x over previous
"""Trainium2 Bass kernel for nn_MultiHeadAttention_85375359909998.

Causal MHA with (non-standard interleaved) RoPE, fp32 in/out.
  B=2, T=2048, D=1024, H=16, DH=64.

Sharding over 8 NeuronCores: data-parallel over batch (2) x tensor-parallel
over head groups (16 heads -> 4 groups of 4). Each core computes its batch's
QKV projection for its 4 heads, RoPE, causal attention, and a partial output
projection; the host sums the 4 partial projections per batch (the
"all-reduce") and concatenates batches.

Device-side layout notes (per core, heads grouped in pairs):
  - PE operands are fp16 (1 cycle/column streaming + fast weight loads);
    accumulation stays fp32 in PSUM. Measured end-to-end error vs the
    fp32 reference: max-rel ~5e-4, resid_var ~6e-7.
  - q/k are produced *transposed* ([dh, t]) directly by the projection
    (host passes x^T and W^T); RoPE rotate-half is a 128x128 block-diagonal
    permutation matrix applied on the PE, combined with cos/sin muls split
    across DVE and GpSimd; evictions ride the Scalar engine
  - scores are computed transposed (S^T[s, t]) so the A@V matmul can use
    P^T tiles as the moving operand with V ([s, dh]) stationary; V gets an
    appended ones-column so row-sums (softmax denominators) fall out of the
    same matmul; causal masking = skipping upper tiles/columns + an
    accumulating identity-matmul that adds a precomputed -inf pattern to
    the diagonal blocks
  - softmax normalization: DVE fast-reciprocal of the sums row, GpSimd
    partition-broadcast, DVE multiply.
"""

import sys
from contextlib import ExitStack

import numpy as np

try:
    import concourse.bass as bass  # noqa: F401
except ImportError:  # pragma: no cover
    sys.path.insert(0, "/opt/trn_rl_repo")
    import concourse.bass as bass  # noqa: F401

import concourse.tile as tile
from concourse import bacc, mybir
from concourse import bass_utils

B, T, D, H, DH = 2, 2048, 1024, 16, 64
NCORES = 8
GROUPS = 4          # head groups (tensor-parallel dimension)
HPC = H // GROUPS   # 4 heads per core
NPAIR = HPC // 2    # head pairs per core
TC512 = T // 512    # 4
SC128 = T // 128    # 16
KC = D // 128       # 8 contraction chunks for the projections
NEG8 = np.float32(-4.0e4)  # pre-scale additive mask: exp(NEG8/8) == 0, fits fp16

f32 = mybir.dt.float32
f32r = mybir.dt.float32r
f16 = mybir.dt.float16
EXP = mybir.ActivationFunctionType.Exp
COPY = mybir.ActivationFunctionType.Copy

PAT_OFF = [0, 128, 384, 768]
PAT_W = [128, 256, 384, 512]

_CACHE = {}


def _rope_tables():
    """cos/sin tables, transposed & stacked for the [2*64, t] chunk layout."""
    inv = 1.0 / (10000.0 ** (np.arange(0, DH, 2, dtype=np.float64) / DH))  # 32
    t = np.arange(T, dtype=np.float64)
    freqs = t[:, None] * inv[None, :]                 # [T, 32]
    emb = np.concatenate([freqs, freqs], axis=-1)     # [T, 64]
    cos = np.cos(emb).astype(np.float32).T            # [64, T]
    sin = np.sin(emb).astype(np.float32).T
    csc = np.concatenate([cos, cos], axis=0)          # [128, T]
    csn = np.concatenate([sin, sin], axis=0)
    return (np.ascontiguousarray(csc.astype(np.float16)),
            np.ascontiguousarray(csn.astype(np.float16)))


def _rot_matrix():
    """R.T for rotate_half: (R@v)[2i] = -v[2i+1], (R@v)[2i+1] = v[2i]."""
    R = np.zeros((DH, DH), dtype=np.float32)
    for i in range(DH // 2):
        R[2 * i, 2 * i + 1] = -1.0
        R[2 * i + 1, 2 * i] = 1.0
    R128 = np.zeros((128, 128), dtype=np.float32)
    R128[:DH, :DH] = R
    R128[DH:, DH:] = R
    return np.ascontiguousarray(R128.T)


def _diag_patterns():
    """Concatenated diagonal-block causal masks, widths 128/256/384/512."""
    blocks = []
    for r in range(4):
        w = 128 * (r + 1)
        sp = np.arange(128)[:, None]
        tp = np.arange(w)[None, :]
        blocks.append(np.where(tp < 128 * r + sp, NEG8, np.float32(0.0)))
    return np.ascontiguousarray(
        np.concatenate(blocks, axis=1).astype(np.float16))


def _emit(nc, tc, d, ctx):
    const = ctx.enter_context(tc.tile_pool(name="const", bufs=1))
    qkp = ctx.enter_context(tc.tile_pool(name="qk", bufs=1))
    vtp = ctx.enter_context(tc.tile_pool(name="vt", bufs=1))
    vp = ctx.enter_context(tc.tile_pool(name="v", bufs=1))
    att = ctx.enter_context(tc.tile_pool(name="att", bufs=1))
    ptp = ctx.enter_context(tc.tile_pool(name="pt", bufs=3))
    tmp = ctx.enter_context(tc.tile_pool(name="tmp", bufs=3))
    small = ctx.enter_context(tc.tile_pool(name="small", bufs=2))
    stage = ctx.enter_context(tc.tile_pool(name="stage", bufs=4))

    # ---- constants ----
    wqk_t, wv_t = [], []
    for kc in range(KC):
        w1 = const.tile([128, 512], f16, tag=f"wqk{kc}")
        nc.scalar.dma_start(w1[:], d["wqk"][128 * kc:128 * (kc + 1), :])
        wqk_t.append(w1)
        w2 = const.tile([128, 256], f16, tag=f"wv{kc}")
        nc.scalar.dma_start(w2[:], d["wv"][128 * kc:128 * (kc + 1), :])
        wv_t.append(w2)
    rT_t = const.tile([128, 128], f16, tag="rT")
    nc.scalar.dma_start(rT_t[:], d["rT"][:])
    csc_t = const.tile([128, T], f16, tag="csc")
    csn_t = const.tile([128, T], f16, tag="csn")
    id_t = const.tile([128, 128], f16, tag="ident")
    oc_t = const.tile([128, 1], f16, tag="onesc")
    pat_t = const.tile([128, 1280], f16, tag="pat")
    wp_t = [const.tile([128, D], f16, tag=f"wp{kc}", name=f"wp{kc}")
            for kc in range(2)]

    # ---- persistent activations ----
    xT_t = []
    for kc in range(KC):
        xt_ = const.tile([128, T], f16, tag=f"xt{kc}", name=f"xt{kc}")
        dma_eng = nc.sync if kc % 2 == 0 else nc.scalar
        dma_eng.dma_start(xt_[:], d["xT"][128 * kc:128 * (kc + 1), :])
        xT_t.append(xt_)

    qk = [qkp.tile([128, T], f16, tag=f"qk{j}", name=f"qk{j}")
          for j in range(4)]
    vt = [vtp.tile([128, T], f16, tag=f"vt{j}", name=f"vt{j}")
          for j in range(2)]
    v_sb = [vp.tile([128, 4 * 65], f16, tag=f"v{i}", name=f"v{i}")
            for i in range(SC128)]
    attn = [att.tile([128, T], f16, tag=f"at{p}", name=f"at{p}")
            for p in range(NPAIR)]

    # ===== phase 1: qkv projections (transposed) + RoPE + V transpose ==
    with tc.tile_pool(name="psA", bufs=4, space="PSUM") as psA, \
         tc.tile_pool(name="psR", bufs=2, space="PSUM") as psR, \
         tc.tile_pool(name="psT", bufs=2, space="PSUM") as psT:
        for tcc in range(TC512):
            tsl = slice(512 * tcc, 512 * (tcc + 1))
            xst = [xT_t[kc][:, tsl] for kc in range(KC)]
            if tcc == 0:
                # lower-priority constants: needed only from the RoPE /
                # transpose / attention / proj stages onward
                nc.gpsimd.dma_start(csc_t[:], d["csc"][:])
                nc.gpsimd.dma_start(csn_t[:], d["csn"][:])
                nc.gpsimd.dma_start(id_t[:], d["ident"][:])
                nc.gpsimd.dma_start(oc_t[:], d["onesc"][:])
            if tcc == 1:
                nc.gpsimd.dma_start(pat_t[:], d["pat"][:])
                for kc2 in range(2):
                    nc.gpsimd.dma_start(
                        wp_t[kc2][:], d["wp"][128 * kc2:128 * (kc2 + 1), :])
            for jp in range(3):
                pss2 = [psA.tile([128, 512], f32, tag="ps", name=f"ps{jc}")
                        for jc in (2 * jp, 2 * jp + 1)]
                for kc in range(KC):
                    for u in range(2):
                        jc = 2 * jp + u
                        lhsT = (wqk_t[kc][:, 128 * jc:128 * (jc + 1)]
                                if jc < 4 else
                                wv_t[kc][:, 128 * (jc - 4):128 * (jc - 3)])
                        nc.tensor.matmul(pss2[u][:], lhsT, xst[kc],
                                         start=(kc == 0), stop=(kc == KC - 1))
                for u in range(2):
                    jc = 2 * jp + u
                    ps = pss2[u]
                    if jc < 4:
                        dst = qk[jc][:, tsl]
                        nc.scalar.copy(dst, ps[:])
                        rps = psR.tile([128, 512], f32, tag="rps")
                        nc.tensor.matmul(rps[:], rT_t[:], dst,
                                         start=True, stop=True)
                        t1 = tmp.tile([128, 512], f32, tag="t1")
                        nc.vector.tensor_mul(t1[:], rps[:], csn_t[:, tsl])
                        t2 = tmp.tile([128, 512], f32, tag="t2")
                        nc.gpsimd.tensor_mul(t2[:], dst, csc_t[:, tsl])
                        nc.vector.tensor_add(dst, t1[:], t2[:])
                    else:
                        nc.scalar.copy(vt[jc - 4][:, tsl], ps[:])
            # transpose this t-chunk's v^T into v [s, dh] + ones columns
            for b4 in range(4):
                i = 4 * tcc + b4
                vdst = v_sb[i]
                for vc in range(2):
                    pst = psT.tile([128, 128], f16, tag="pst")
                    nc.tensor.transpose(
                        pst[:], vt[vc][:, 128 * i:128 * (i + 1)], id_t[:])
                    ha = 2 * vc
                    dst3 = vdst[:, 65 * ha:65 * ha + 130].rearrange(
                        "p (h e) -> p h e", h=2, e=65)[:, :, 0:64]
                    src3 = pst[:].rearrange("p (h e) -> p h e", h=2, e=64)
                    nc.vector.tensor_copy(dst3, src3)
                nc.vector.tensor_copy(vdst[:, 64:260:65],
                                      oc_t[:, 0:1].broadcast_to([128, 4]))

    # ===== phase 3: causal attention + interleaved projection ======
    with tc.tile_pool(name="psS", bufs=2, space="PSUM") as psS, \
         tc.tile_pool(name="psO", bufs=1, space="PSUM") as psO, \
         tc.tile_pool(name="psP", bufs=2, space="PSUM") as psP:
        for j in range(TC512):
            tsl = slice(512 * j, 512 * (j + 1))
            ni = 4 * (j + 1)          # causal s-chunks for this strip
            for p in range(NPAIR):
                qc = qk[2 * p]
                kch = qk[2 * p + 1]
                po = [psO.tile([65, 512], f32, tag=f"o{hh}", name=f"po{hh}")
                      for hh in range(2)]
                for g in range(ni // 2):
                    pss = [psS.tile([128, 1024], f32, tag="s",
                                    name=f"pss{hh}") for hh in range(2)]
                    for half in range(2):
                        ii = 2 * g + half
                        diag = ii >= 4 * j
                        r = ii - 4 * j if diag else 0
                        c0 = 512 * half + (128 * r if diag else 0)
                        qs = slice(512 * j + 128 * r, 512 * (j + 1)) \
                            if diag else tsl
                        for hh in range(2):
                            hsl = slice(64 * hh, 64 * (hh + 1))
                            nc.tensor.matmul(
                                pss[hh][:, c0:512 * half + 512],
                                kch[hsl, 128 * ii:128 * (ii + 1)],
                                qc[hsl, qs],
                                start=True, stop=not diag)
                        if diag:
                            w = PAT_W[r]
                            for hh in range(2):
                                nc.tensor.matmul(
                                    pss[hh][:, 512 * half:512 * half + w],
                                    id_t[:],
                                    pat_t[:, PAT_OFF[r]:PAT_OFF[r] + w],
                                    start=False, stop=True)
                    ptl = [ptp.tile([128, 1024], f16, tag=f"ptl{hh}",
                                    name=f"ptl{hh}") for hh in range(2)]
                    for hh in range(2):
                        nc.scalar.activation(ptl[hh][:], pss[hh][:], EXP,
                                             scale=0.125)
                    for half in range(2):
                        ii = 2 * g + half
                        diag = ii >= 4 * j
                        r = ii - 4 * j if diag else 0
                        c0 = 128 * r if diag else 0
                        for hh in range(2):
                            h = 2 * p + hh
                            nc.tensor.matmul(
                                po[hh][:, c0:512],
                                v_sb[ii][:, 65 * h:65 * h + 65],
                                ptl[hh][:, 512 * half + c0:512 * half + 512],
                                start=(ii == 0), stop=(ii == ni - 1))
                for hh in range(2):
                    sr = small.tile([1, 512], f32, tag="bsb")
                    nc.vector.tensor_copy(sr[:], po[hh][64:65, :])
                    rc = small.tile([1, 512], f32, tag="rc")
                    nc.vector.reciprocal_approx_fast(rc[:], sr[:])
                    bsb = small.tile([64, 512], f32, tag="bsb")
                    nc.gpsimd.partition_broadcast(bsb[:], rc[0:1, :])
                    nc.vector.tensor_mul(
                        attn[p][64 * hh:64 * (hh + 1), tsl],
                        po[hh][0:64, :], bsb[:])
            # projection for this finished t-strip overlaps later strips
            for oc in range(D // 128):
                pp = psP.tile([128, 512], f32, tag="pp")
                for kc2 in range(2):
                    nc.tensor.matmul(
                        pp[:], wp_t[kc2][:, 128 * oc:128 * (oc + 1)],
                        attn[kc2][:, tsl],
                        start=(kc2 == 0), stop=(kc2 == 1))
                ob = stage.tile([128, 512], f32, tag="ob")
                if oc % 2 == 0:
                    nc.vector.tensor_copy(ob[:], pp[:])
                else:
                    nc.scalar.activation(ob[:], pp[:], COPY)
                nc.sync.dma_start(d["o"][128 * oc:128 * (oc + 1), tsl], ob[:])


def _build_module():
    nc = bacc.Bacc("TRN2", target_bir_lowering=False, debug=False,
                   enable_asserts=False)
    d = {
        "xT": nc.dram_tensor("xT", [D, T], f16, kind="ExternalInput").ap(),
        "wqk": nc.dram_tensor("wqk", [D, 512], f16, kind="ExternalInput").ap(),
        "wv": nc.dram_tensor("wv", [D, 256], f16, kind="ExternalInput").ap(),
        "wp": nc.dram_tensor("wp", [256, D], f16, kind="ExternalInput").ap(),
        "csc": nc.dram_tensor("csc", [128, T], f16, kind="ExternalInput").ap(),
        "csn": nc.dram_tensor("csn", [128, T], f16, kind="ExternalInput").ap(),
        "rT": nc.dram_tensor("rT", [128, 128], f16, kind="ExternalInput").ap(),
        "ident": nc.dram_tensor("ident", [128, 128], f16,
                                kind="ExternalInput").ap(),
        "ones1": nc.dram_tensor("ones1", [1, 64], f16,
                                kind="ExternalInput").ap(),
        "onesc": nc.dram_tensor("onesc", [128, 1], f16,
                                kind="ExternalInput").ap(),
        "pat": nc.dram_tensor("pat", [128, 1280], f16,
                              kind="ExternalInput").ap(),
        "o": nc.dram_tensor("o", [D, T], f32, kind="ExternalOutput").ap(),
    }
    with tile.TileContext(nc) as tc:
        with ExitStack() as ctx, \
             nc.allow_low_precision("fp32r PE operands are rounded by design"):
            _emit(nc, tc, d, ctx)
    nc.compile()
    return nc


def _get_module():
    if "nc" not in _CACHE:
        _CACHE["nc"] = _build_module()
    return _CACHE["nc"]


def _canonical(attn_mask, key_padding_mask):
    if attn_mask.shape != (1, 1, T, T) or key_padding_mask.shape != (B, T):
        return False
    if not key_padding_mask.all():
        return False
    m = np.asarray(attn_mask[0, 0], dtype=np.float32)
    causal = np.triu(np.full((T, T), -1e9, dtype=np.float32), k=1)
    return np.array_equal(m, causal)


def _reference_fallback(x, attn_mask, key_padding_mask, Wqkv, Wproj):
    x = np.asarray(x, np.float32)
    qkv = x @ np.asarray(Wqkv, np.float32).T
    q, k, v = qkv[..., :D], qkv[..., D:2 * D], qkv[..., 2 * D:]

    def split(t):
        return t.reshape(B, -1, H, DH).transpose(0, 2, 1, 3)

    def rope(xx):
        inv = 1.0 / (10000.0 ** (np.arange(0, DH, 2, dtype=np.float32) / DH))
        fr = np.arange(T, dtype=np.float32)[:, None] * inv[None, :]
        emb = np.concatenate([fr, fr], axis=-1)
        cos, sin = np.cos(emb)[None, None], np.sin(emb)[None, None]
        x1, x2 = xx[..., ::2], xx[..., 1::2]
        rh = np.stack((-x2, x1), axis=-1).reshape(xx.shape)
        return xx * cos + rh * sin

    q, k, v = split(q), split(k), split(v)
    q, k = rope(q), rope(k)
    s = np.einsum("bhtd,bhsd->bhts", q, k) / np.sqrt(np.float32(DH))
    s = s + np.asarray(attn_mask, np.float32)
    s = np.where(np.asarray(key_padding_mask)[:, None, None, :], s, -1e9)
    s = s - s.max(axis=-1, keepdims=True)
    e = np.exp(s)
    a = e / e.sum(axis=-1, keepdims=True)
    out = np.einsum("bhts,bhsd->bhtd", a, v)
    out = out.transpose(0, 2, 1, 3).reshape(B, T, D)
    return out @ np.asarray(Wproj, np.float32).T


def _make_in_maps(x, Wqkv, Wproj):
    csc, csn = _rope_tables()
    rT = _rot_matrix().astype(np.float16)
    pat = _diag_patterns()
    ident = np.eye(128, dtype=np.float16)
    ones1 = np.ones((1, 64), dtype=np.float16)
    onesc = np.ones((128, 1), dtype=np.float16)

    Wq = np.asarray(Wqkv[:D], np.float32).reshape(H, DH, D)
    Wk = np.asarray(Wqkv[D:2 * D], np.float32).reshape(H, DH, D)
    Wv = np.asarray(Wqkv[2 * D:], np.float32).reshape(H, DH, D)
    WpT = np.ascontiguousarray(np.asarray(Wproj, np.float32).T)  # [din, dout]

    xT = [np.ascontiguousarray(np.asarray(x[b], np.float32).T.astype(np.float16))
          for b in range(B)]

    in_maps = []
    for c in range(NCORES):
        b, g = divmod(c, GROUPS)
        hs = [HPC * g + hl for hl in range(HPC)]  # global head ids
        cols = []
        for pp in range(NPAIR):
            h0, h1 = hs[2 * pp], hs[2 * pp + 1]
            cols.append(np.concatenate([Wq[h0], Wq[h1]], axis=0))  # [128, D]
            cols.append(np.concatenate([Wk[h0], Wk[h1]], axis=0))
        wqk = np.ascontiguousarray(
            np.concatenate(cols, axis=0).T.astype(np.float16))     # [D, 512]
        wv = np.ascontiguousarray(
            np.concatenate([Wv[h] for h in hs], axis=0).T.astype(np.float16))
        wp = np.ascontiguousarray(
            WpT[256 * g:256 * (g + 1), :].astype(np.float16))  # [256, D]
        in_maps.append({
            "xT": xT[b], "wqk": wqk, "wv": wv, "wp": wp,
            "csc": csc, "csn": csn, "rT": rT, "ident": ident,
            "ones1": ones1, "onesc": onesc, "pat": pat,
        })
    return in_maps


def _in_maps_for_trace(inputs):
    return _make_in_maps(np.asarray(inputs["x"]), np.asarray(inputs["Wqkv"]),
                         np.asarray(inputs["Wproj"]))


def kernel(x, attn_mask, key_padding_mask, Wqkv, Wproj):
    x = np.asarray(x)
    attn_mask = np.asarray(attn_mask)
    key_padding_mask = np.asarray(key_padding_mask)
    Wqkv = np.asarray(Wqkv)
    Wproj = np.asarray(Wproj)

    if not _canonical(attn_mask, key_padding_mask):
        return _reference_fallback(x, attn_mask, key_padding_mask, Wqkv, Wproj)

    nc = _get_module()
    in_maps = _make_in_maps(x, Wqkv, Wproj)
    res = bass_utils.run_bass_kernel_spmd(nc, in_maps,
                                          core_ids=list(range(NCORES)))
    out = np.empty((B, T, D), dtype=np.float32)
    for b in range(B):
        acc = res.results[4 * b]["o"].astype(np.float32).copy()
        for g in range(1, GROUPS):
            acc += res.results[4 * b + g]["o"]
        out[b] = acc.T
    return out
